# revision 54
# baseline (speedup 1.0000x reference)
"""CLIP-style loss kernel for Trainium2 (8 NeuronCores, SPMD data-parallel).

Problem: two patch-embeddings (stride-4 4x4 conv -> L2 normalize) of
imgs/hha [32,64,128,128], per-sample logits = exp(logit_scale) * a @ h^T
[B,1024,1024], symmetric cross-entropy with diagonal labels, scalar loss.

Sharding: data-parallel over batch, 4 samples per core. Each core reads only
its 4x2 images (32MB -> memory-bound), produces per-sample partial sums
(row-lse, col-lse, diag); host combines in float64.

Per-core device pipeline (per sample b, modality m in {a=imgs, h=hha}):
  conv:   im2col-free conv as 16 accumulating K=64 matmuls per patch-half.
          Image is loaded as [128, 8192]: partitions (t,c) = (row-half,
          channel), so the two patch-halves run on disjoint PE row groups
          (rows 0-63 / 64-127) => concurrent matmuls, and the DMA uses all
          128 partitions (full HBM bandwidth). Y = [d, n] in PSUM.
  norms:  sq = Y*Y (DVE). a-side: norm2 in [n-part, chunk] layout via 8
          matmuls (lhsT=sq chunk, rhs=ones) -> rsqrt via Ln->Exp with the
          logit scale folded in (ACT Rsqrt is banned for accuracy).
          h-side: norm2 in [1, n] layout via ones-matmul column sums ->
          Ln->Exp -> broadcast to [128, n] via K=1 matmul -> h_hat = Y*inv.
  logits: chunk k: matmul(lhsT=Ya[:,k*128:...], rhs=h_hat) -> L [128,1024].
          exp is UNstabilized (|logit| <= exp(logit_scale) ~ 14.3, safe in
          fp32) with the a-side row scale applied via the ACT per-partition
          scale operand, and the row-sum fused via accum_out.
  colsum: ones-matmul over E chunks accumulating in PSUM -> [1, 1024].
  diag:   t = Ya .* h_hat; per-chunk column sums via matmul; dot with invT.
Output per core: [128, 12] partial-sum block; host reduces.
"""

import os
import sys
from contextlib import ExitStack

import numpy as np

for _p in ("/opt/trn_rl_repo", "/root/.axon_site/_ro/trn_rl_repo"):
    if os.path.isdir(_p) and _p not in sys.path:
        sys.path.insert(0, _p)

import concourse.bass as bass
import concourse.mybir as mybir
import concourse.tile as tile
from concourse import bacc
from concourse.bass_utils import run_bass_kernel_spmd

F32 = mybir.dt.float32
AF = mybir.ActivationFunctionType
ALU = mybir.AluOpType

N_CORES = 8
B_FULL = 32
BPC = B_FULL // N_CORES  # samples per core
C, H, W, D, P = 64, 128, 128, 128, 4
NPAT = (H // P) * (W // P)  # 1024 patches
NH = NPAT // 2  # 512 (one patch-half / one PSUM bank)
NOFF = P * P  # 16 kernel offsets
NCHUNK = NPAT // 128  # 8 logit row chunks


def build_program(ln_s: float) -> bass.Bass:
    # dev bisect knob: 1=conv only, 2=+norms, 3=+logits/exp/csum, 4=full
    stage = int(os.environ.get("BASS_KERNEL_STAGE", "6"))
    nc = bacc.Bacc(None)
    imgs = nc.declare_dram_parameter("imgs", [BPC, C, H, W], F32, isOutput=False)
    hha = nc.declare_dram_parameter("hha", [BPC, C, H, W], F32, isOutput=False)
    w1t = nc.declare_dram_parameter("w1t", [C, NOFF, D], F32, isOutput=False)
    w2t = nc.declare_dram_parameter("w2t", [C, NOFF, D], F32, isOutput=False)
    b1 = nc.declare_dram_parameter("b1", [D], F32, isOutput=False)
    b2 = nc.declare_dram_parameter("b2", [D], F32, isOutput=False)
    out_d = nc.declare_dram_parameter("out", [128, 3 * BPC], F32, isOutput=True)

    # [(row-half, chan), (row-in-half * width)] view for full-128-partition DMA
    def img_src(src, b):
        base = src[b]
        return bass.AP(
            tensor=base.tensor,
            offset=base.offset,
            ap=[[(H // 2) * W, 2], [H * W, C], [1, (H // 2) * W]],
        )

    srcs = (imgs, hha)

    with tile.TileContext(nc) as tc, ExitStack() as ctx:
        # SBUF pools
        p_img = ctx.enter_context(tc.tile_pool(name="img", bufs=3))
        p_one = ctx.enter_context(tc.tile_pool(name="singles", bufs=1))
        p_ysb = ctx.enter_context(tc.tile_pool(name="ysb", bufs=3))
        p_sq = ctx.enter_context(tc.tile_pool(name="sq", bufs=2))
        p_hhat = ctx.enter_context(tc.tile_pool(name="hhat", bufs=2))
        p_E = ctx.enter_context(tc.tile_pool(name="E", bufs=3))
        p_cs = ctx.enter_context(tc.tile_pool(name="cs", bufs=2))
        p_sm = ctx.enter_context(tc.tile_pool(name="small", bufs=2))
        # PSUM pools (8 banks total: 3 + 2 + 1 + 2)
        pp512 = ctx.enter_context(tc.tile_pool(name="pp512", bufs=3, space="PSUM"))
        pp_T = ctx.enter_context(tc.tile_pool(name="ppT", bufs=2, space="PSUM"))
        pp_wu = ctx.enter_context(tc.tile_pool(name="ppwu", bufs=1, space="PSUM"))

        # constants / weights (loaded once)
        ones_k = p_one.tile([128, 1], F32)  # colsum lhsT & norm rhs
        nc.vector.memset(ones_k, 1.0)
        ones_m = p_one.tile([1, 128], F32)  # broadcast lhsT (K=1)
        nc.vector.memset(ones_m, 1.0)
        wts = []
        biases = []
        for wsrc, bsrc in ((w1t, b1), (w2t, b2)):
            # one DMA for both partition-half copies (walrus allows only one
            # sync wait per matmul, so the first weight reader must depend on
            # a single transfer)
            wt = p_one.tile([128, NOFF, D], F32, tag=f"wt_{wsrc.name}")
            wsrc_dup = bass.AP(
                tensor=wsrc[:].tensor,
                offset=wsrc[:].offset,
                ap=[[0, 2], [NOFF * D, C], [D, NOFF], [1, D]],
            )
            nc.sync.dma_start(out=wt, in_=wsrc_dup)
            wts.append(wt)
            bt = p_one.tile([128, 1], F32, tag=f"bias_{bsrc.name}")
            nc.sync.dma_start(out=bt, in_=bsrc[:].rearrange("(d one) -> d one", one=1))
            biases.append(bt)
        OUT = p_one.tile([128, 3 * BPC], F32)
        nc.vector.memset(OUT, 0.0)
        lns = p_one.tile([128, 1], F32)  # ln(logit scale) bias for a-side rsqrt
        nc.vector.memset(lns, ln_s)

        # Warmup matmuls: absorb the weight-DMA waits into PE program order so
        # the first conv matmul only waits on its image DMA (walrus allows a
        # single sync wait per matmul). All warmup/absorber matmuls write
        # disjoint columns of ONE persistent PSUM tile — no pool-slot reuse,
        # so none of them inherits slot-release deps (extra waits).
        wu_all = pp_wu.tile([128, NH], F32)
        # persistent [1, N] tile for the h-side norm2 row (rewritten per
        # sample; same-tensor PE rewrites are FIFO-safe, so no extra waits)
        n2f_all = pp_wu.tile([1, NPAT], F32)
        for m in range(2):
            nc.tensor.matmul(
                wu_all[:, BPC + m : BPC + m + 1],
                wts[m][0:64, 0, :],
                wts[m][0:64, 0, 0:1],
                start=True,
                stop=True,
            )

        E_last = None
        for b in range(BPC):
            wub_inst = None
            if b > 0 and stage >= 3:
                # Wait-absorber: one tiny matmul reading the previous sample's
                # last exp output advances PE's observed ACT clock, so this
                # sample's matmuls' PSUM-slot-release deps (ACT reader ticks)
                # are elided and each matmul keeps <=1 wait.
                wub_inst = nc.tensor.matmul(
                    wu_all[:, b : b + 1],
                    wts[0][0:64, 0, :],
                    E_last[0:64, NPAT - 1 : NPAT],
                    start=True,
                    stop=True,
                )
            y_sb = {}
            invT = None  # a-side row scales [128(n-chunk part), 8]
            h_hat = None
            for m in range(2):
                # ---- load image (one full-width DMA, 4MB) ----
                img = p_img.tile([128, (H // 2) * W], F32, tag="img")
                nc.sync.dma_start(out=img, in_=img_src(srcs[m], b))
                # [p, i(16), di(4), j(32), dj(4)] conv view
                iv = img.rearrange(
                    "p (i di j dj) -> p i di j dj", i=16, di=4, j=32, dj=4
                )

                # ---- conv: 2 patch-halves x 16 offsets, K=64 row-tiled ----
                conv = [
                    pp512.tile([128, NH], F32, tag="p512", name=f"conv_{b}_{m}_{t}")
                    for t in range(2)
                ]
                for o in range(NOFF):
                    di, dj = o // 4, o % 4
                    for t in range(2):
                        mm = nc.tensor.matmul(
                            conv[t],
                            wts[m][64 * t : 64 * t + 64, o, :],
                            iv[64 * t : 64 * t + 64, :, di, :, dj],
                            start=(o == 0),
                            stop=(o == NOFF - 1),
                        )
                        if wub_inst is not None:
                            # order the absorber before the conv (no semaphore)
                            tile.add_dep_helper(
                                mm.ins, wub_inst.ins, sync=False,
                                reason="wait-absorber ordering",
                            )
                            wub_inst = None

                # ---- Y -> SBUF (+bias) ----
                ym = p_ysb.tile([128, NPAT], F32, tag="ysb")
                for t in range(2):
                    nc.scalar.activation(
                        out=ym[:, t * NH : (t + 1) * NH],
                        in_=conv[t],
                        func=AF.Identity,
                        bias=biases[m],
                    )
                y_sb[m] = ym

                # ---- squared ----
                sq = p_sq.tile([128, NPAT], F32, tag="sq")
                nc.vector.tensor_mul(sq, ym, ym)

                if stage < 2:
                    continue

                if m == 0:
                    # a-side: norm2 in [n-part, chunk] layout (bank-padded tile
                    # so the two pp_T slots never share a PSUM bank)
                    n2T = pp_T.tile([128, NH], F32, tag="ppT", name=f"n2T_{b}")
                    # DVE-absorber: pre-observe the sq tick on PE so the first
                    # n2T matmul's only wait is the PSUM bank-reuse drain
                    nc.tensor.matmul(
                        wu_all[:, 6 + 3 * b : 7 + 3 * b],
                        wts[m][0:64, 0, :],
                        sq[0:64, 0:1],
                        start=True,
                        stop=True,
                    )
                    for k in range(NCHUNK):
                        nc.tensor.matmul(
                            n2T[:, k : k + 1],
                            sq[:, 128 * k : 128 * (k + 1)],
                            ones_k,
                            start=True,
                            stop=True,
                        )
                    uT = p_sm.tile([128, NCHUNK], F32, tag="uT")
                    nc.scalar.activation(out=uT, in_=n2T[:, 0:NCHUNK], func=AF.Ln)
                    invT = p_sm.tile([128, NCHUNK], F32, tag="invT")
                    # exp(-0.5*ln(n2) + ln_s) = s / sqrt(n2)
                    nc.scalar.activation(
                        out=invT, in_=uT, func=AF.Exp, scale=-0.5, bias=lns
                    )
                else:
                    # h-side: norm2 along partitions via ones-matmul into the
                    # persistent [1, N] tile (free-dim layout feeds the bcast)
                    for j in range(2):
                        nc.tensor.matmul(
                            n2f_all[:, j * NH : (j + 1) * NH],
                            ones_k,
                            sq[:, j * NH : (j + 1) * NH],
                            start=True,
                            stop=True,
                        )
                    uf = p_sm.tile([1, NPAT], F32, tag="uf")
                    nc.scalar.activation(out=uf, in_=n2f_all[0:1, :], func=AF.Ln)
                    invf = p_sm.tile([1, NPAT], F32, tag="invf")
                    nc.scalar.activation(
                        out=invf, in_=uf, func=AF.Exp, scale=-0.5
                    )
                    h_hat = p_hhat.tile([128, NPAT], F32, tag="hhat")
                    for j in range(2):
                        bc = pp512.tile([128, NH], F32, tag="p512")
                        nc.tensor.matmul(
                            bc,
                            ones_m,
                            invf[:, j * NH : (j + 1) * NH],
                            start=True,
                            stop=True,
                        )
                        nc.vector.tensor_mul(
                            h_hat[:, j * NH : (j + 1) * NH],
                            ym[:, j * NH : (j + 1) * NH],
                            bc,
                        )

            if stage < 3:
                continue

            # ---- logits chunks: exp + fused rowsum; colsum accumulation ----
            RS = p_sm.tile([128, 2 * NCHUNK], F32, tag="RS")
            csum = p_cs.tile([128, NPAT], F32, tag="cs")
            for k in range(NCHUNK):
                E = p_E.tile([128, NPAT], F32, tag="E", name=f"E_{b}_{k}")
                if k == NCHUNK - 1:
                    E_last = E
                for j in range(2):
                    L = pp512.tile([128, NH], F32, tag="p512")
                    nc.tensor.matmul(
                        L,
                        y_sb[0][:, 128 * k : 128 * (k + 1)],
                        h_hat[:, j * NH : (j + 1) * NH],
                        start=True,
                        stop=True,
                    )
                    nc.scalar.activation(
                        out=E[:, j * NH : (j + 1) * NH],
                        in_=L,
                        func=AF.Exp,
                        scale=invT[:, k : k + 1],
                        accum_out=RS[:, 2 * k + j : 2 * k + j + 1],
                    )
                # accumulate exp columns on DVE (partition-reduce at the end)
                if k == 0:
                    nc.vector.tensor_copy(csum, E)
                else:
                    nc.vector.tensor_add(csum, csum, E)

            if stage < 4:
                continue

            # ---- row lse: add half-row sums, ln, accumulate into OUT ----
            RS2 = p_sm.tile([128, NCHUNK], F32, tag="RS2")
            RSv = RS.rearrange("p (k two) -> p k two", two=2)
            nc.vector.tensor_add(RS2, RSv[:, :, 0], RSv[:, :, 1])
            lseR = p_sm.tile([128, NCHUNK], F32, tag="lseR")
            nc.scalar.activation(
                out=lseR, in_=RS2, func=AF.Ln, accum_out=OUT[:, b : b + 1]
            )

            if stage < 5:
                continue

            # ---- col lse: partition-reduce the E accumulator into T-layout
            # via per-chunk ones-matmuls, ln + fused per-partition sum; the
            # host adds up the 128 partial sums.
            nc.tensor.matmul(
                wu_all[:, 8 + 3 * b : 9 + 3 * b],
                wts[0][0:64, 0, :],
                csum[0:64, 0:1],
                start=True,
                stop=True,
            )
            csT = pp_T.tile([128, NH], F32, tag="ppT", name=f"csT_{b}")
            for q in range(NCHUNK):
                nc.tensor.matmul(
                    csT[:, q : q + 1],
                    csum[:, 128 * q : 128 * (q + 1)],
                    ones_k,
                    start=True,
                    stop=True,
                )
            lseC = p_sm.tile([128, NCHUNK], F32, tag="lseC")
            nc.scalar.activation(
                out=lseC,
                in_=csT[:, 0:NCHUNK],
                func=AF.Ln,
                accum_out=OUT[:, 2 * BPC + b : 2 * BPC + b + 1],
            )

            if stage < 6:
                continue

            # ---- diag: t = Ya .* h_hat; per-chunk col sums; dot with invT ----
            t_ = p_sq.tile([128, NPAT], F32, tag="sq")
            nc.vector.tensor_mul(t_, y_sb[0], h_hat)
            cT = pp_T.tile([128, NH], F32, tag="ppT", name=f"cT_{b}")
            nc.tensor.matmul(
                wu_all[:, 7 + 3 * b : 8 + 3 * b],
                wts[0][0:64, 0, :],
                t_[0:64, 0:1],
                start=True,
                stop=True,
            )
            for k in range(NCHUNK):
                nc.tensor.matmul(
                    cT[:, k : k + 1],
                    t_[:, 128 * k : 128 * (k + 1)],
                    ones_k,
                    start=True,
                    stop=True,
                )
            dg = p_sm.tile([128, NCHUNK], F32, tag="dg")
            nc.vector.tensor_mul(dg, cT[:, 0:NCHUNK], invT)
            nc.vector.tensor_reduce(
                out=OUT[:, BPC + b : BPC + b + 1],
                in_=dg,
                axis=mybir.AxisListType.X,
                op=ALU.add,
            )

        nc.sync.dma_start(out=out_d[:], in_=OUT)

    nc.compile()
    return nc


_PROGRAM_CACHE: dict = {}


def _get_program(ln_s: float) -> bass.Bass:
    key = round(float(ln_s), 9)
    if key not in _PROGRAM_CACHE:
        _PROGRAM_CACHE[key] = build_program(float(ln_s))
    return _PROGRAM_CACHE[key]


def make_in_maps(imgs, hha, w1, b1, w2, b2):
    """Shard full inputs into per-core input maps (host-side, cheap)."""
    # weights -> [C, (di dj), D] contiguous so the device DMA is trivial
    w1t = np.ascontiguousarray(np.transpose(np.asarray(w1), (1, 2, 3, 0))).reshape(
        C, NOFF, D
    )
    w2t = np.ascontiguousarray(np.transpose(np.asarray(w2), (1, 2, 3, 0))).reshape(
        C, NOFF, D
    )
    imgs = np.asarray(imgs)
    hha = np.asarray(hha)
    b1 = np.ascontiguousarray(np.asarray(b1), dtype=np.float32)
    b2 = np.ascontiguousarray(np.asarray(b2), dtype=np.float32)
    maps = []
    for i in range(N_CORES):
        maps.append(
            {
                "imgs": np.ascontiguousarray(imgs[i * BPC : (i + 1) * BPC]),
                "hha": np.ascontiguousarray(hha[i * BPC : (i + 1) * BPC]),
                "w1t": w1t,
                "w2t": w2t,
                "b1": b1,
                "b2": b2,
            }
        )
    return maps


def combine_outputs(outs) -> np.float32:
    """Reduce the 8 per-core [128, 3*BPC] partial blocks to the scalar loss."""
    tot = np.float64(0.0)
    for o in outs:
        o = np.asarray(o, dtype=np.float64)
        lse_row = o[:, 0:BPC].sum()
        diag = o[:, BPC : 2 * BPC].sum()
        lse_col = o[:, 2 * BPC : 3 * BPC].sum()
        tot += 0.5 * (lse_row + lse_col) - diag
    return np.float32(tot / (B_FULL * NPAT))


def run_spmd(imgs, hha, w1, b1, w2, b2, logit_scale, **kwargs):
    """Run on the 8 cores; returns (loss, BassKernelResults)."""
    ln_s = float(np.asarray(logit_scale))
    nc = _get_program(ln_s)
    in_maps = make_in_maps(imgs, hha, w1, b1, w2, b2)
    res = run_bass_kernel_spmd(nc, in_maps, list(range(N_CORES)), **kwargs)
    return combine_outputs([r["out"] for r in res.results]), res


def kernel(imgs, hha, w1, b1, w2, b2, logit_scale):
    loss, _ = run_spmd(imgs, hha, w1, b1, w2, b2, logit_scale)
    return loss


if __name__ == "__main__":
    # smoke test against a tiny numpy reference of the math
    rng = np.random.default_rng(0)
    imgs = rng.standard_normal((B_FULL, C, H, W), dtype=np.float32)
    hha = rng.standard_normal((B_FULL, C, H, W), dtype=np.float32)
    w1 = rng.standard_normal((D, C, P, P), dtype=np.float32) * 0.03
    w2 = rng.standard_normal((D, C, P, P), dtype=np.float32) * 0.03
    b1 = np.zeros(D, np.float32)
    b2 = np.zeros(D, np.float32)
    ls = np.float32(np.log(1.0 / 0.07))
    print(kernel(imgs, hha, w1, b1, w2, b2, ls))


# revision 55
# speedup vs baseline: 1.3137x; 1.3137x over previous
"""CLIP-style loss kernel for Trainium2 (8 NeuronCores, SPMD data-parallel).

Problem: two patch-embeddings (stride-4 4x4 conv -> L2 normalize) of
imgs/hha [32,64,128,128], per-sample logits = exp(logit_scale) * a @ h^T
[B,1024,1024], symmetric cross-entropy with diagonal labels, scalar loss.

Sharding: data-parallel over batch, 4 samples per core. Each core reads only
its 4x2 images (32MB -> memory-bound), produces per-sample partial sums
(row-lse, col-lse, diag); host combines in float64.

Per-core device pipeline (per sample b, modality m in {a=imgs, h=hha}):
  conv:   im2col-free conv as 16 accumulating K=64 matmuls per patch-half.
          Image is loaded as [128, 8192]: partitions (t,c) = (row-half,
          channel), so the two patch-halves run on disjoint PE row groups
          (rows 0-63 / 64-127) => concurrent matmuls, and the DMA uses all
          128 partitions (full HBM bandwidth). Y = [d, n] in PSUM.
  norms:  sq = Y*Y (DVE). a-side: norm2 in [n-part, chunk] layout via 8
          matmuls (lhsT=sq chunk, rhs=ones) -> rsqrt via Ln->Exp with the
          logit scale folded in (ACT Rsqrt is banned for accuracy).
          h-side: norm2 in [1, n] layout via ones-matmul column sums ->
          Ln->Exp -> broadcast to [128, n] via K=1 matmul -> h_hat = Y*inv.
  logits: chunk k: matmul(lhsT=Ya[:,k*128:...], rhs=h_hat) -> L [128,1024].
          exp is UNstabilized (|logit| <= exp(logit_scale) ~ 14.3, safe in
          fp32) with the a-side row scale applied via the ACT per-partition
          scale operand, and the row-sum fused via accum_out.
  colsum: ones-matmul over E chunks accumulating in PSUM -> [1, 1024].
  diag:   t = Ya .* h_hat; per-chunk column sums via matmul; dot with invT.
Output per core: [128, 12] partial-sum block; host reduces.
"""

import os
import sys
from contextlib import ExitStack

import numpy as np

for _p in ("/opt/trn_rl_repo", "/root/.axon_site/_ro/trn_rl_repo"):
    if os.path.isdir(_p) and _p not in sys.path:
        sys.path.insert(0, _p)

import concourse.bass as bass
import concourse.mybir as mybir
import concourse.tile as tile
from concourse import bacc
from concourse.bass_utils import run_bass_kernel_spmd

F32 = mybir.dt.float32
AF = mybir.ActivationFunctionType
ALU = mybir.AluOpType

N_CORES = 8
B_FULL = 32
BPC = B_FULL // N_CORES  # samples per core
C, H, W, D, P = 64, 128, 128, 128, 4
NPAT = (H // P) * (W // P)  # 1024 patches
NH = NPAT // 2  # 512 (one patch-half / one PSUM bank)
NOFF = P * P  # 16 kernel offsets
NCHUNK = NPAT // 128  # 8 logit row chunks


def build_program(ln_s: float) -> bass.Bass:
    # dev bisect knob: 1=conv only, 2=+norms, 3=+logits/exp/csum, 4=full
    stage = int(os.environ.get("BASS_KERNEL_STAGE", "6"))
    nc = bacc.Bacc(None)
    imgs = nc.declare_dram_parameter("imgs", [BPC, C, H, W], F32, isOutput=False)
    hha = nc.declare_dram_parameter("hha", [BPC, C, H, W], F32, isOutput=False)
    w1t = nc.declare_dram_parameter("w1t", [C, NOFF, D], F32, isOutput=False)
    w2t = nc.declare_dram_parameter("w2t", [C, NOFF, D], F32, isOutput=False)
    b1 = nc.declare_dram_parameter("b1", [D], F32, isOutput=False)
    b2 = nc.declare_dram_parameter("b2", [D], F32, isOutput=False)
    out_d = nc.declare_dram_parameter("out", [128, 3 * BPC], F32, isOutput=True)

    # [(row-half, chan), (row-in-half * width)] view for full-128-partition DMA
    def img_src(src, b):
        base = src[b]
        return bass.AP(
            tensor=base.tensor,
            offset=base.offset,
            ap=[[(H // 2) * W, 2], [H * W, C], [1, (H // 2) * W]],
        )

    srcs = (imgs, hha)

    with tile.TileContext(nc) as tc, ExitStack() as ctx:
        # SBUF pools
        p_img = ctx.enter_context(tc.tile_pool(name="img", bufs=3))
        p_one = ctx.enter_context(tc.tile_pool(name="singles", bufs=1))
        p_ysb = ctx.enter_context(tc.tile_pool(name="ysb", bufs=3))
        p_sq = ctx.enter_context(tc.tile_pool(name="sq", bufs=2))
        p_hhat = ctx.enter_context(tc.tile_pool(name="hhat", bufs=2))
        p_E = ctx.enter_context(tc.tile_pool(name="E", bufs=3))
        p_cs = ctx.enter_context(tc.tile_pool(name="cs", bufs=2))
        p_sm = ctx.enter_context(tc.tile_pool(name="small", bufs=2))
        # PSUM pools (8 banks total: 3 + 2 + 1 + 2)
        pp512 = ctx.enter_context(tc.tile_pool(name="pp512", bufs=3, space="PSUM"))
        pp_T = ctx.enter_context(tc.tile_pool(name="ppT", bufs=2, space="PSUM"))
        pp_wu = ctx.enter_context(tc.tile_pool(name="ppwu", bufs=1, space="PSUM"))

        # constants / weights (loaded once)
        ones_k = p_one.tile([128, 1], F32)  # colsum lhsT & norm rhs
        nc.vector.memset(ones_k, 1.0)
        ones_m = p_one.tile([1, 128], F32)  # broadcast lhsT (K=1)
        nc.vector.memset(ones_m, 1.0)
        wts = []
        biases = []
        for wsrc, bsrc in ((w1t, b1), (w2t, b2)):
            # one DMA for both partition-half copies (walrus allows only one
            # sync wait per matmul, so the first weight reader must depend on
            # a single transfer)
            wt = p_one.tile([128, NOFF, D], F32, tag=f"wt_{wsrc.name}")
            wsrc_dup = bass.AP(
                tensor=wsrc[:].tensor,
                offset=wsrc[:].offset,
                ap=[[0, 2], [NOFF * D, C], [D, NOFF], [1, D]],
            )
            nc.sync.dma_start(out=wt, in_=wsrc_dup)
            wts.append(wt)
            bt = p_one.tile([128, 1], F32, tag=f"bias_{bsrc.name}")
            nc.sync.dma_start(out=bt, in_=bsrc[:].rearrange("(d one) -> d one", one=1))
            biases.append(bt)
        OUT = p_one.tile([128, 3 * BPC], F32)
        nc.vector.memset(OUT, 0.0)
        lns = p_one.tile([128, 1], F32)  # ln(logit scale) bias for a-side rsqrt
        nc.vector.memset(lns, ln_s)

        # Warmup matmuls: absorb the weight-DMA waits into PE program order so
        # the first conv matmul only waits on its image DMA (walrus allows a
        # single sync wait per matmul). All warmup/absorber matmuls write
        # disjoint columns of ONE persistent PSUM tile — no pool-slot reuse,
        # so none of them inherits slot-release deps (extra waits).
        wu_all = pp_wu.tile([128, NH], F32)
        # persistent [1, N] tile for the h-side norm2 row (rewritten per
        # sample; same-tensor PE rewrites are FIFO-safe, so no extra waits)
        n2f_all = pp_wu.tile([1, NPAT], F32)
        for m in range(2):
            nc.tensor.matmul(
                wu_all[:, BPC + m : BPC + m + 1],
                wts[m][0:64, 0, :],
                wts[m][0:64, 0, 0:1],
                start=True,
                stop=True,
            )

        E_last = None
        for b in range(BPC):
            wub_inst = None
            if b > 0 and stage >= 3:
                # Wait-absorber: one tiny matmul reading the previous sample's
                # last exp output advances PE's observed ACT clock, so this
                # sample's matmuls' PSUM-slot-release deps (ACT reader ticks)
                # are elided and each matmul keeps <=1 wait.
                wub_inst = nc.tensor.matmul(
                    wu_all[:, b : b + 1],
                    wts[0][0:64, 0, :],
                    E_last[0:64, NPAT - 1 : NPAT],
                    start=True,
                    stop=True,
                )
            y_sb = {}
            invT = None  # a-side row scales [128(n-chunk part), 8]
            h_hat = None
            for m in range(2):
                # ---- load image (one full-width DMA, 4MB) ----
                img = p_img.tile([128, (H // 2) * W], F32, tag="img")
                nc.gpsimd.dma_start(out=img, in_=img_src(srcs[m], b))
                # [p, i(16), di(4), j(32), dj(4)] conv view
                iv = img.rearrange(
                    "p (i di j dj) -> p i di j dj", i=16, di=4, j=32, dj=4
                )

                # ---- conv: 2 patch-halves x 16 offsets, K=64 row-tiled ----
                conv = [
                    pp512.tile([128, NH], F32, tag="p512", name=f"conv_{b}_{m}_{t}")
                    for t in range(2)
                ]
                for o in range(NOFF):
                    di, dj = o // 4, o % 4
                    for t in range(2):
                        mm = nc.tensor.matmul(
                            conv[t],
                            wts[m][64 * t : 64 * t + 64, o, :],
                            iv[64 * t : 64 * t + 64, :, di, :, dj],
                            start=(o == 0),
                            stop=(o == NOFF - 1),
                        )
                        if wub_inst is not None:
                            # order the absorber before the conv (no semaphore)
                            tile.add_dep_helper(
                                mm.ins, wub_inst.ins, sync=False,
                                reason="wait-absorber ordering",
                            )
                            wub_inst = None

                # ---- Y -> SBUF (+bias) ----
                ym = p_ysb.tile([128, NPAT], F32, tag="ysb")
                for t in range(2):
                    nc.scalar.activation(
                        out=ym[:, t * NH : (t + 1) * NH],
                        in_=conv[t],
                        func=AF.Identity,
                        bias=biases[m],
                    )
                y_sb[m] = ym

                # ---- squared ----
                sq = p_sq.tile([128, NPAT], F32, tag="sq")
                nc.vector.tensor_mul(sq, ym, ym)

                if stage < 2:
                    continue

                if m == 0:
                    # a-side: norm2 in [n-part, chunk] layout (bank-padded tile
                    # so the two pp_T slots never share a PSUM bank)
                    n2T = pp_T.tile([128, NH], F32, tag="ppT", name=f"n2T_{b}")
                    # DVE-absorber: pre-observe the sq tick on PE so the first
                    # n2T matmul's only wait is the PSUM bank-reuse drain
                    nc.tensor.matmul(
                        wu_all[:, 6 + 3 * b : 7 + 3 * b],
                        wts[m][0:64, 0, :],
                        sq[0:64, 0:1],
                        start=True,
                        stop=True,
                    )
                    for k in range(NCHUNK):
                        nc.tensor.matmul(
                            n2T[:, k : k + 1],
                            sq[:, 128 * k : 128 * (k + 1)],
                            ones_k,
                            start=True,
                            stop=True,
                        )
                    uT = p_sm.tile([128, NCHUNK], F32, tag="uT")
                    nc.scalar.activation(out=uT, in_=n2T[:, 0:NCHUNK], func=AF.Ln)
                    invT = p_sm.tile([128, NCHUNK], F32, tag="invT")
                    # exp(-0.5*ln(n2) + ln_s) = s / sqrt(n2)
                    nc.scalar.activation(
                        out=invT, in_=uT, func=AF.Exp, scale=-0.5, bias=lns
                    )
                else:
                    # h-side: norm2 along partitions via ones-matmul into the
                    # persistent [1, N] tile (free-dim layout feeds the bcast)
                    for j in range(2):
                        nc.tensor.matmul(
                            n2f_all[:, j * NH : (j + 1) * NH],
                            ones_k,
                            sq[:, j * NH : (j + 1) * NH],
                            start=True,
                            stop=True,
                        )
                    uf = p_sm.tile([1, NPAT], F32, tag="uf")
                    nc.scalar.activation(out=uf, in_=n2f_all[0:1, :], func=AF.Ln)
                    invf = p_sm.tile([1, NPAT], F32, tag="invf")
                    nc.scalar.activation(
                        out=invf, in_=uf, func=AF.Exp, scale=-0.5
                    )
                    h_hat = p_hhat.tile([128, NPAT], F32, tag="hhat")
                    for j in range(2):
                        bc = pp512.tile([128, NH], F32, tag="p512")
                        nc.tensor.matmul(
                            bc,
                            ones_m,
                            invf[:, j * NH : (j + 1) * NH],
                            start=True,
                            stop=True,
                        )
                        nc.vector.tensor_mul(
                            h_hat[:, j * NH : (j + 1) * NH],
                            ym[:, j * NH : (j + 1) * NH],
                            bc,
                        )

            if stage < 3:
                continue

            # ---- logits chunks: exp + fused rowsum; colsum accumulation ----
            RS = p_sm.tile([128, 2 * NCHUNK], F32, tag="RS")
            csum = p_cs.tile([128, NPAT], F32, tag="cs")
            for k in range(NCHUNK):
                E = p_E.tile([128, NPAT], F32, tag="E", name=f"E_{b}_{k}")
                if k == NCHUNK - 1:
                    E_last = E
                for j in range(2):
                    L = pp512.tile([128, NH], F32, tag="p512")
                    nc.tensor.matmul(
                        L,
                        y_sb[0][:, 128 * k : 128 * (k + 1)],
                        h_hat[:, j * NH : (j + 1) * NH],
                        start=True,
                        stop=True,
                    )
                    nc.scalar.activation(
                        out=E[:, j * NH : (j + 1) * NH],
                        in_=L,
                        func=AF.Exp,
                        scale=invT[:, k : k + 1],
                        accum_out=RS[:, 2 * k + j : 2 * k + j + 1],
                    )
                # accumulate exp columns on DVE (partition-reduce at the end)
                if k == 0:
                    nc.vector.tensor_copy(csum, E)
                else:
                    nc.vector.tensor_add(csum, csum, E)

            if stage < 4:
                continue

            # ---- row lse: add half-row sums, ln, accumulate into OUT ----
            RS2 = p_sm.tile([128, NCHUNK], F32, tag="RS2")
            RSv = RS.rearrange("p (k two) -> p k two", two=2)
            nc.vector.tensor_add(RS2, RSv[:, :, 0], RSv[:, :, 1])
            lseR = p_sm.tile([128, NCHUNK], F32, tag="lseR")
            nc.scalar.activation(
                out=lseR, in_=RS2, func=AF.Ln, accum_out=OUT[:, b : b + 1]
            )

            if stage < 5:
                continue

            # ---- col lse: partition-reduce the E accumulator into T-layout
            # via per-chunk ones-matmuls, ln + fused per-partition sum; the
            # host adds up the 128 partial sums.
            nc.tensor.matmul(
                wu_all[:, 8 + 3 * b : 9 + 3 * b],
                wts[0][0:64, 0, :],
                csum[0:64, 0:1],
                start=True,
                stop=True,
            )
            csT = pp_T.tile([128, NH], F32, tag="ppT", name=f"csT_{b}")
            for q in range(NCHUNK):
                nc.tensor.matmul(
                    csT[:, q : q + 1],
                    csum[:, 128 * q : 128 * (q + 1)],
                    ones_k,
                    start=True,
                    stop=True,
                )
            lseC = p_sm.tile([128, NCHUNK], F32, tag="lseC")
            nc.scalar.activation(
                out=lseC,
                in_=csT[:, 0:NCHUNK],
                func=AF.Ln,
                accum_out=OUT[:, 2 * BPC + b : 2 * BPC + b + 1],
            )

            if stage < 6:
                continue

            # ---- diag: t = Ya .* h_hat; per-chunk col sums; dot with invT ----
            t_ = p_sq.tile([128, NPAT], F32, tag="sq")
            nc.vector.tensor_mul(t_, y_sb[0], h_hat)
            cT = pp_T.tile([128, NH], F32, tag="ppT", name=f"cT_{b}")
            nc.tensor.matmul(
                wu_all[:, 7 + 3 * b : 8 + 3 * b],
                wts[0][0:64, 0, :],
                t_[0:64, 0:1],
                start=True,
                stop=True,
            )
            for k in range(NCHUNK):
                nc.tensor.matmul(
                    cT[:, k : k + 1],
                    t_[:, 128 * k : 128 * (k + 1)],
                    ones_k,
                    start=True,
                    stop=True,
                )
            dg = p_sm.tile([128, NCHUNK], F32, tag="dg")
            nc.vector.tensor_mul(dg, cT[:, 0:NCHUNK], invT)
            nc.vector.tensor_reduce(
                out=OUT[:, BPC + b : BPC + b + 1],
                in_=dg,
                axis=mybir.AxisListType.X,
                op=ALU.add,
            )

        nc.sync.dma_start(out=out_d[:], in_=OUT)

    nc.compile()
    return nc


_PROGRAM_CACHE: dict = {}


def _get_program(ln_s: float) -> bass.Bass:
    key = round(float(ln_s), 9)
    if key not in _PROGRAM_CACHE:
        _PROGRAM_CACHE[key] = build_program(float(ln_s))
    return _PROGRAM_CACHE[key]


def make_in_maps(imgs, hha, w1, b1, w2, b2):
    """Shard full inputs into per-core input maps (host-side, cheap)."""
    # weights -> [C, (di dj), D] contiguous so the device DMA is trivial
    w1t = np.ascontiguousarray(np.transpose(np.asarray(w1), (1, 2, 3, 0))).reshape(
        C, NOFF, D
    )
    w2t = np.ascontiguousarray(np.transpose(np.asarray(w2), (1, 2, 3, 0))).reshape(
        C, NOFF, D
    )
    imgs = np.asarray(imgs)
    hha = np.asarray(hha)
    b1 = np.ascontiguousarray(np.asarray(b1), dtype=np.float32)
    b2 = np.ascontiguousarray(np.asarray(b2), dtype=np.float32)
    maps = []
    for i in range(N_CORES):
        maps.append(
            {
                "imgs": np.ascontiguousarray(imgs[i * BPC : (i + 1) * BPC]),
                "hha": np.ascontiguousarray(hha[i * BPC : (i + 1) * BPC]),
                "w1t": w1t,
                "w2t": w2t,
                "b1": b1,
                "b2": b2,
            }
        )
    return maps


def combine_outputs(outs) -> np.float32:
    """Reduce the 8 per-core [128, 3*BPC] partial blocks to the scalar loss."""
    tot = np.float64(0.0)
    for o in outs:
        o = np.asarray(o, dtype=np.float64)
        lse_row = o[:, 0:BPC].sum()
        diag = o[:, BPC : 2 * BPC].sum()
        lse_col = o[:, 2 * BPC : 3 * BPC].sum()
        tot += 0.5 * (lse_row + lse_col) - diag
    return np.float32(tot / (B_FULL * NPAT))


def run_spmd(imgs, hha, w1, b1, w2, b2, logit_scale, **kwargs):
    """Run on the 8 cores; returns (loss, BassKernelResults)."""
    ln_s = float(np.asarray(logit_scale))
    nc = _get_program(ln_s)
    in_maps = make_in_maps(imgs, hha, w1, b1, w2, b2)
    res = run_bass_kernel_spmd(nc, in_maps, list(range(N_CORES)), **kwargs)
    return combine_outputs([r["out"] for r in res.results]), res


def kernel(imgs, hha, w1, b1, w2, b2, logit_scale):
    loss, _ = run_spmd(imgs, hha, w1, b1, w2, b2, logit_scale)
    return loss


if __name__ == "__main__":
    # smoke test against a tiny numpy reference of the math
    rng = np.random.default_rng(0)
    imgs = rng.standard_normal((B_FULL, C, H, W), dtype=np.float32)
    hha = rng.standard_normal((B_FULL, C, H, W), dtype=np.float32)
    w1 = rng.standard_normal((D, C, P, P), dtype=np.float32) * 0.03
    w2 = rng.standard_normal((D, C, P, P), dtype=np.float32) * 0.03
    b1 = np.zeros(D, np.float32)
    b2 = np.zeros(D, np.float32)
    ls = np.float32(np.log(1.0 / 0.07))
    print(kernel(imgs, hha, w1, b1, w2, b2, ls))
